# revision 3
# baseline (speedup 1.0000x reference)
"""CapsuleLayer dynamic-routing kernel for 8 Trainium2 NeuronCores.

Problem (hardcoded shapes):
  x [512, 1152, 8] f32, W [10, 1152, 8, 16] f32
  priors = einsum('bri,nrio->nbro'); 3 rounds of softmax-over-R routing.
  out [10, 512, 1, 1, 16] f32.

Sharding: 4-way batch (128 each) x 2-way capsule N (5 each) over 8 cores.

Strategy: priors are NEVER materialized in [b, o, r] layout. All state stays
in the transposed layout xT [(r16, i) partition, (rb, b) free] where every
contraction is a PE matmul:

  logit update   l[b,r] = sum_o P[b,r,o] v[b,o] = sum_i x[b,r,i] G[b,r,i],
                 G = W @ v        -> per-rb matmuls (K=16, lhsT=W^T slices)
                 sum_i            -> one matmul with block matrix
                                     S = kron(I16, ones(8,8))
  weighted sum   su[b,o] = sum_{r,i} (e*x)[b,(r,i)] W[(r,i),o]
                                   -> accumulating matmuls (K=128)
  softmax Z      Z[b] = sum_r e    -> accumulating 1-col matmuls vs 0.125
                                      (e is stored 8x i-replicated)

Logits are linear in v, so round 3 reuses the same pipeline with
vsum = v1 + v2 instead of keeping l1 around. Remaining elementwise work per
capsule: two q = xT*G muls (DVE, PSUM f32 operand) and two y = e*xT muls
(DVE scalar_tensor_tensor, 4x bf16 mode). exp on ACT straight out of the
S-matmul's PSUM chunks; sqrt inside squash is exp(0.5*ln(x)) so ACT never
leaves the natural_log_exp_and_others table.

The per-round g-loop (18 chunks of 4 r-blocks) is software-pipelined with
lag stages (GT ahead, Z/su one behind) to keep PE/DVE/ACT all busy.
"""

import numpy as np

B, R, I, O, N = 512, 1152, 8, 16, 10
BG, NG = 4, 2              # batch groups x capsule groups = 8 cores
BL, NL = B // BG, N // NG  # 128, 5
RB = R // 16               # 72 r-blocks of 16
CH = 4                     # r-blocks per chunk
G = RB // CH               # 18 chunks
NCORES = 8

_CACHE = {}


def _build_program(debug=False, repeat=1):
    import concourse.tile as tile
    from concourse import bacc, mybir

    F32 = mybir.dt.float32
    BF16 = mybir.dt.bfloat16
    ALU = mybir.AluOpType
    ACTF = mybir.ActivationFunctionType

    nc = bacc.Bacc("TRN2", target_bir_lowering=False, debug=debug,
                   num_devices=NCORES)

    xT_d = nc.dram_tensor("xT", [128, RB * BL], BF16, kind="ExternalInput")
    WT_d = nc.dram_tensor("WT", [NL, 16, RB * BL], BF16, kind="ExternalInput")
    wd_d = nc.dram_tensor("wd", [NL, 128, RB * O], BF16, kind="ExternalInput")
    S_d = nc.dram_tensor("S", [128, 128], BF16, kind="ExternalInput")
    I_d = nc.dram_tensor("Ident", [128, 128], BF16, kind="ExternalInput")
    on_d = nc.dram_tensor("ones8", [128, 1], BF16, kind="ExternalInput")
    out_d = nc.dram_tensor("out", [128, NL * O], F32, kind="ExternalOutput")

    with tile.TileContext(nc) as tc:
        with (
            tc.tile_pool(name="const", bufs=1) as cpool,
            tc.tile_pool(name="chunk", bufs=3) as chpool,
            tc.tile_pool(name="small", bufs=4) as smpool,
            tc.tile_pool(name="psbig", bufs=3, space="PSUM") as psb,
            tc.tile_pool(name="psl", bufs=2, space="PSUM") as psl,
            tc.tile_pool(name="pssu", bufs=2, space="PSUM") as pssu,
            tc.tile_pool(name="psvt", bufs=1, space="PSUM") as psvt,
        ):
            xT = cpool.tile([128, RB * BL], BF16, tag="xT")
            S = cpool.tile([128, 128], BF16, tag="S")
            Ident = cpool.tile([128, 128], BF16, tag="Ident")
            ones8 = cpool.tile([128, 1], BF16, tag="ones8")
            outacc = cpool.tile([128, NL * O], F32, tag="outacc")
            wt = [cpool.tile([16, RB * BL], BF16, name=f"wt{j}")
                  for j in range(2)]
            wdn = [cpool.tile([128, RB * O], BF16, name=f"wdn{j}")
                   for j in range(2)]
            vsum = [cpool.tile([16, 128], BF16, name=f"vsum{j}")
                    for j in range(2)]

            nc.sync.dma_start(xT[:], xT_d[:])
            nc.sync.dma_start(S[:], S_d[:])
            nc.sync.dma_start(Ident[:], I_d[:])
            nc.sync.dma_start(ones8[:], on_d[:])

            def squash(su_ap, z_recip_ap, dst_v):
                # v = s * sqrt(n2) / (1 + n2), s = su / Z, n2 = sum_o s^2
                # sqrt(n2) = exp(0.5 * ln(n2)): keeps ACT on the exp/ln table
                s = smpool.tile([128, O], F32, tag="s")
                if z_recip_ap is None:
                    nc.vector.tensor_scalar_mul(s[:], su_ap, 1.0 / R)
                else:
                    nc.vector.tensor_scalar_mul(s[:], su_ap, z_recip_ap)
                sqj = smpool.tile([128, O], F32, tag="sqj")
                n2 = smpool.tile([128, 1], F32, tag="n2")
                nc.scalar.activation(sqj[:], s[:], ACTF.Square, accum_out=n2[:])
                la = smpool.tile([128, 1], F32, tag="la")
                nc.scalar.activation(la[:], n2[:], ACTF.Ln)
                rt = smpool.tile([128, 1], F32, tag="rt")
                nc.scalar.activation(rt[:], la[:], ACTF.Exp, scale=0.5)
                u = smpool.tile([128, 1], F32, tag="u")
                nc.vector.tensor_scalar_add(u[:], n2[:], 1.0)
                rr = smpool.tile([128, 1], F32, tag="rr")
                nc.vector.reciprocal(rr[:], u[:])
                sc = smpool.tile([128, 1], F32, tag="sc")
                nc.vector.tensor_mul(sc[:], rt[:], rr[:])
                nc.vector.tensor_scalar_mul(dst_v, s[:], sc[:])

            def vsum_set(v_bf, par, add):
                # transpose v [128 b, 16 o] -> [16 o, 128 b] on PE, then
                # either copy into vsum or accumulate (logits linear in v)
                vt = psvt.tile([16, 128], BF16, tag="vt")
                nc.tensor.transpose(vt[:], v_bf, Ident[:])
                if add:
                    nc.vector.tensor_add(vsum[par][:], vsum[par][:], vt[:])
                else:
                    nc.vector.tensor_copy(vsum[par][:], vt[:])

            def su1_pass(par):
                su = pssu.tile([128, 32], F32, tag="su")
                for rb in range(RB):
                    nc.tensor.matmul(
                        su[:, 0:O], xT[:, rb * BL:(rb + 1) * BL],
                        wdn[par][:, rb * O:(rb + 1) * O],
                        start=(rb == 0), stop=(rb == RB - 1),
                        skip_group_check=True)
                return su

            def round_pass(par, t):
                """Fused logit-update + weighted-sum pass over 18 chunks,
                software-pipelined: GT(g) | q/S/e(g-1) | y/Z/su(g-2)."""
                su = pssu.tile([128, 32], F32, tag="su")
                gts = {}
                qs = {}
                es = {}

                def emit_gt(g):
                    gt = psb.tile([128, CH * BL], F32, tag="gt")
                    for j in range(CH):
                        rb = g * CH + j
                        nc.tensor.matmul(
                            gt[:, j * BL:(j + 1) * BL],
                            wt[par][:, rb * BL:(rb + 1) * BL],
                            vsum[par][:],
                            start=True, stop=True, skip_group_check=True)
                    gts[g] = gt

                def emit_qse(g):
                    gsl = slice(g * CH * BL, (g + 1) * CH * BL)
                    q = chpool.tile([128, CH * BL], BF16, tag="q")
                    # GPSIMD takes a third of the PSUM-operand muls
                    qeng = nc.gpsimd if g % 3 == 2 else nc.vector
                    qeng.tensor_tensor(q[:], xT[:, gsl], gts.pop(g)[:],
                                       op=ALU.mult)
                    l = psl.tile([128, CH * BL], F32, tag="l")
                    nc.tensor.matmul(l[:], S[:], q[:],
                                     start=True, stop=True,
                                     skip_group_check=True)
                    e = chpool.tile([128, CH * BL], BF16, tag="e")
                    nc.scalar.activation(e[:], l[:], ACTF.Exp)
                    qs[g] = q
                    es[g] = e

                def emit_yzsu(g):
                    gsl = slice(g * CH * BL, (g + 1) * CH * BL)
                    e = es.pop(g)
                    qs.pop(g)
                    y = chpool.tile([128, CH * BL], BF16, tag="y")
                    yeng = nc.gpsimd if g % 3 == 1 else nc.vector
                    yeng.tensor_tensor(y[:], e[:], xT[:, gsl], op=ALU.mult)
                    first = g == 0
                    last = g == G - 1
                    for j in range(CH):
                        rb = g * CH + j
                        nc.tensor.matmul(
                            su[:, 16:17], e[:, j * BL:(j + 1) * BL],
                            ones8[:],
                            start=(first and j == 0), stop=(last and j == CH - 1),
                            skip_group_check=True)
                        nc.tensor.matmul(
                            su[:, 0:O], y[:, j * BL:(j + 1) * BL],
                            wdn[par][:, rb * O:(rb + 1) * O],
                            start=(first and j == 0), stop=(last and j == CH - 1),
                            skip_group_check=True)

                for g in range(G + 2):
                    if g < G:
                        emit_gt(g)
                    if 0 <= g - 1 < G:
                        emit_qse(g - 1)
                    if 0 <= g - 2 < G:
                        emit_yzsu(g - 2)
                return su

            NT = repeat * NL
            for nr in range(NT):
                n = nr % NL
                par = nr % 2
                # split the 16-partition WT DMA so 8 queues run in parallel
                for h in range(8):
                    sl = slice(h * (RB * BL // 8), (h + 1) * (RB * BL // 8))
                    nc.sync.dma_start(wt[par][:, sl], WT_d[n][:, sl])
                nc.sync.dma_start(wdn[par][:], wd_d[n])

                su1 = su1_pass(par)
                v1 = smpool.tile([128, O], BF16, tag="v1")
                squash(su1[:, 0:O], None, v1[:])
                vsum_set(v1[:], par, add=False)

                su2 = round_pass(par, 2)
                rz2 = smpool.tile([128, 1], F32, tag="rz2")
                nc.vector.reciprocal(rz2[:], su2[:, 16:17])
                v2 = smpool.tile([128, O], BF16, tag="v2")
                squash(su2[:, 0:O], rz2[:], v2[:])
                vsum_set(v2[:], par, add=True)

                su3 = round_pass(par, 3)
                rz3 = smpool.tile([128, 1], F32, tag="rz3")
                nc.vector.reciprocal(rz3[:], su3[:, 16:17])
                squash(su3[:, 0:O], rz3[:], outacc[:, n * O:(n + 1) * O])

            nc.sync.dma_start(out_d[:], outacc[:])

    nc.compile()
    return nc


def _host_prep(x, W):
    """Build per-core input maps (bf16, transposed layouts)."""
    from ml_dtypes import bfloat16

    x = np.ascontiguousarray(x, dtype=np.float32)
    W = np.ascontiguousarray(W, dtype=np.float32)
    S = np.kron(np.eye(16, dtype=np.float32),
                np.ones((8, 8), np.float32)).astype(bfloat16)
    Ident = np.eye(128, dtype=np.float32).astype(bfloat16)
    ones8 = np.full((128, 1), 0.125, np.float32).astype(bfloat16)
    in_maps = []
    for c in range(NCORES):
        bg, ng = c % BG, c // BG
        xs = x[bg * BL:(bg + 1) * BL]                      # [128, 1152, 8]
        # xT[(r16*8+i), rb*128+b] = xs[b, rb*16+r16, i]
        xT = np.ascontiguousarray(
            xs.reshape(BL, RB, 16, 8).transpose(2, 3, 1, 0)
            .reshape(128, RB * BL)).astype(bfloat16)
        Wn = W[ng * NL:(ng + 1) * NL].reshape(NL, RB, 16, 8, O)
        # WT[n][o, rb*128+(r16*8+i)] = W[n, rb*16+r16, i, o]
        WT = np.ascontiguousarray(
            Wn.transpose(0, 4, 1, 2, 3).reshape(NL, O, RB * 128)).astype(bfloat16)
        # wd[n][(r16*8+i), rb*16+o] = W[n, rb*16+r16, i, o]
        wd = np.ascontiguousarray(
            Wn.transpose(0, 2, 3, 1, 4).reshape(NL, 128, RB * O)).astype(bfloat16)
        in_maps.append({"xT": xT, "WT": WT, "wd": wd,
                        "S": S, "Ident": Ident, "ones8": ones8})
    return in_maps


def _gather(results):
    out = np.zeros((N, B, 1, 1, O), np.float32)
    for c in range(NCORES):
        bg, ng = c % BG, c // BG
        o = results[c]["out"].reshape(BL, NL, O)           # [b, n, o]
        out[ng * NL:(ng + 1) * NL, bg * BL:(bg + 1) * BL, 0, 0, :] = \
            o.transpose(1, 0, 2)
    return out


def kernel(x, W):
    from concourse.bass_utils import run_bass_kernel_spmd
    if "nc" not in _CACHE:
        _CACHE["nc"] = _build_program()
    nc = _CACHE["nc"]
    in_maps = _host_prep(x, W)
    res = run_bass_kernel_spmd(nc, in_maps, core_ids=list(range(NCORES)))
    _CACHE["last_results"] = res
    return _gather(res.results)


if __name__ == "__main__":
    d = np.load("/root/problem/work/ref.npz")
    out = kernel(d["x"], d["W"])
    exp = d["expected"]
    rel = np.linalg.norm(out - exp) / np.linalg.norm(exp)
    print("rel err:", rel)
